# revision 38
# baseline (speedup 1.0000x reference)
"""LocallyConnected3D (valid, stride 1) as a Trainium2 Bass kernel on 8 NeuronCores.

Math: out[b,l,f] = sum_p patch[b,l,p] * K[l,p,f] + bias[l,f]
  with B=4, L=27000 output locations, P=216 receptive field, F=16 filters.

The (L,P,F) kernel tensor (373 MB fp32) dominates: each element is used
exactly B=4 times, so the problem is HBM-bandwidth-bound.  Strategy:

- Shard L across the 8 cores (3375 locations each, padded to 3456 = 108
  groups of 32 locations).
- Stream the kernel in fp8-e3m4 scaled by 64 (values land in e3m4's normal
  range; the 1/64 descale is folded into the block-diagonal mask constant)
  and the patches in bf16.  Measured end-to-end rel err ~1.5e-2 against the
  2e-2 gate (verified offline in numpy on the exact inputs).
- Per group g (32 locations), the kernel slab is the *moving* matmul operand:
  psum1[m=(l',b), n=(l,f)] = sum_p patch[b,l',p]*K[l,p,f] via two accumulating
  matmuls (contraction split p=[0,128) and p=[128,216) + one all-ones lhsT row
  carrying 64*bias[l,f]).  Only the block-diagonal l'==l entries are wanted;
  the PE redundancy is free because the kernel byte stream is the bottleneck.
- DVE multiplies psum1 by the bf16 mask (0 or 1/64) into bf16,
- a second matmul (MM2) with a rotating [128,108] selection matrix sums over
  l' AND routes group g's dense [4,512] result into partition slot
  4*(g%27)..+4 of a shared [108,512] PSUM tile, accumulating over a 27-group
  epoch.  One 128-lane scalar-engine copy and one 112-partition output DMA
  per epoch replace 27 lane-starved 4-partition copies/DMAs (4-partition DMAs
  serialize on a single SDMA engine and were the measured critical path).

DMA details (all HW-measured on this part):
- The HW descriptor-generation engine spreads a transfer across all 16 SDMA
  engines ONLY when the SBUF partition count is a multiple of 16; an
  89-partition transfer lands on a single engine and serializes everything.
  Hence chunk 2 is padded to 96 partitions and the staging tile to 48.
- Per-packet (per partition-row) overhead dominates below ~4 KB rows, so
  super-tiles cover SG=12 groups: kernel rows are 6 KB (fp8) and patch rows
  3 KB (bf16), with only 4 streaming transfers per super-tile.

Host-side numpy does the im2col patch extraction and packs kernel/patches
into the exact SBUF tile layouts (index shuffling + dtype cast only - all
FLOPs happen on device).
"""

from contextlib import ExitStack

import ml_dtypes
import numpy as np

import concourse.bacc as bacc
import concourse.mybir as mybir
import concourse.tile as tile
from concourse import bass_utils

F32 = mybir.dt.float32
BF16 = mybir.dt.bfloat16
F8E3 = mybir.dt.float8e3
NP_BF16 = ml_dtypes.bfloat16
NP_F8E3 = ml_dtypes.float8_e3m4

# Geometry (hardcoded per the problem spec)
B, D, H, W, Cin = 4, 32, 32, 32, 8
KD = KH = KW = 3
F = 16
OD = OH = OW = 30
L = OD * OH * OW           # 27000
P = KD * KH * KW * Cin     # 216
NCORE = 8
LC = L // NCORE            # 3375 locations per core
G = 32                     # locations per group
NG = 108                   # groups per core (LC padded to 3456)
LP = NG * G                # 3456
SG = 12                    # groups per DMA super-tile
NSUP = NG // SG            # 9
EP = 27                    # groups per MM2 accumulation epoch
NEP = NG // EP             # 4 output stages
EROW = EP * B              # 108 psum2 partitions per epoch
SROW = 112                 # staged rows (EROW padded to a multiple of 16)
K1 = 128                   # contraction chunk 1 (p in [0,128))
K2 = 96                    # chunk 2: 88 rows p in [128,216) + bias row + 7 pad
KSCALE = 64.0              # fp8 range scale for the kernel stream
NCOL = G * F               # 512 rhs columns per group
MROW = G * B               # 128 psum rows per group

_CACHE = {}


def _build(reps=1):
    nc = bacc.Bacc("TRN2", target_bir_lowering=False, debug=False)

    r1 = nc.dram_tensor("r1", [NSUP, K1, SG * NCOL], F8E3, kind="ExternalInput")
    r2 = nc.dram_tensor("r2", [NSUP, K2, SG * NCOL], F8E3, kind="ExternalInput")
    t1 = nc.dram_tensor("t1", [NSUP, K1, SG * MROW], BF16, kind="ExternalInput")
    t2 = nc.dram_tensor("t2", [NSUP, K2, SG * MROW], F8E3, kind="ExternalInput")
    mask = nc.dram_tensor("mask", [MROW, NCOL], BF16, kind="ExternalInput")
    sel = nc.dram_tensor("sel", [MROW, EP * EROW], BF16, kind="ExternalInput")
    out = nc.dram_tensor("out", [NEP, SROW, NCOL], BF16, kind="ExternalOutput")
    # tiny passthrough token so a timing harness can chain executions
    tok = nc.dram_tensor("tok", [1, 16], F32, kind="ExternalInput")
    tok_out = nc.dram_tensor("tok_out", [1, 16], F32, kind="ExternalOutput")

    with tile.TileContext(nc) as tc, ExitStack() as ctx:
        const_pool = ctx.enter_context(tc.tile_pool(name="const", bufs=1))
        sup_pool = ctx.enter_context(tc.tile_pool(name="sup", bufs=4))
        s_pool = ctx.enter_context(tc.tile_pool(name="s", bufs=5))
        stage_pool = ctx.enter_context(tc.tile_pool(name="stage", bufs=2))
        ps1_pool = ctx.enter_context(tc.tile_pool(name="ps1", bufs=5, space="PSUM"))
        ps2_pool = ctx.enter_context(tc.tile_pool(name="ps2", bufs=2, space="PSUM"))

        # Constants ride the otherwise-idle GpSimd (SWDGE) ring so super 0's
        # kernel/patch streams lead the sync and scalar rings from t=0.
        mask_sb = const_pool.tile([MROW, NCOL], BF16)
        nc.gpsimd.dma_start(mask_sb[:], mask.ap())
        sel_sb = const_pool.tile([MROW, EP * EROW], BF16)
        nc.gpsimd.dma_start(sel_sb[:], sel.ap())
        tok_sb = const_pool.tile([1, 16], F32)
        nc.gpsimd.dma_start(tok_sb[:], tok.ap())
        nc.gpsimd.dma_start(tok_out.ap(), tok_sb[:])

        # Pre-warm the tensor engine while the first super-tile streams in:
        # ~20 dependency-free matmuls on a zeroed scratch tile take the PE
        # through its cold->warm ramp (HAM throttle) so real work starts at
        # full speed.  Results land in a scratch PSUM bank, never read.
        warm_pool = ctx.enter_context(tc.tile_pool(name="warm", bufs=1, space="PSUM"))
        warm_sb = const_pool.tile([MROW, NCOL], BF16)
        nc.vector.memset(warm_sb[:], 0.0)
        psum_w = warm_pool.tile([MROW, NCOL], F32)
        for _ in range(16):
            nc.tensor.matmul(psum_w[:, :MROW], warm_sb[:, :MROW], warm_sb[:, :MROW],
                             start=True, stop=True)

        sup = {}
        psum2 = None

        def emit_tail(g, s_sb):
            # MM2: psum2[4q+b, (l,f)] += sum_{l'} S[(l',b), (l,f)] for q=g%EP,
            # accumulating an epoch of EP groups into one [EROW, NCOL] tile.
            nonlocal psum2
            e, q = g // EP, g % EP
            if q == 0:
                psum2 = ps2_pool.tile([EROW, NCOL], F32)
            nc.tensor.matmul(psum2[:], sel_sb[:, q * EROW:(q + 1) * EROW], s_sb[:],
                             start=(q == 0), stop=(q == EP - 1))
            if q == EP - 1:
                stage_sb = stage_pool.tile([SROW, NCOL], BF16)
                nc.scalar.copy(stage_sb[:EROW, :], psum2[:])
                nc.sync.dma_start(out.ap()[e], stage_sb[:])

        pending = []
        for g_rep in range(reps * NG):
            g = g_rep % NG
            s, j = g // SG, g % SG
            if j == 0:
                sup["r1"] = sup_pool.tile([K1, SG * NCOL], F8E3, tag="r1", name="r1sb")
                nc.sync.dma_start(sup["r1"][:], r1.ap()[s])
                sup["r2"] = sup_pool.tile([K2, SG * NCOL], F8E3, tag="r2", name="r2sb")
                nc.sync.dma_start(sup["r2"][:], r2.ap()[s])
                sup["t1"] = sup_pool.tile([K1, SG * MROW], BF16, tag="t1", name="t1sb")
                nc.scalar.dma_start(sup["t1"][:], t1.ap()[s])
                sup["t2"] = sup_pool.tile([K2, SG * MROW], F8E3, tag="t2", name="t2sb")
                nc.scalar.dma_start(sup["t2"][:], t2.ap()[s])

            psum1 = ps1_pool.tile([MROW, NCOL], F32)
            nc.tensor.matmul(
                psum1[:],
                sup["t1"][:, j * MROW:(j + 1) * MROW],
                sup["r1"][:, j * NCOL:(j + 1) * NCOL],
                start=True, stop=False,
            )
            nc.tensor.matmul(
                psum1[:],
                sup["t2"][:, j * MROW:(j + 1) * MROW],
                sup["r2"][:, j * NCOL:(j + 1) * NCOL],
                start=False, stop=True,
            )
            s_sb = s_pool.tile([MROW, NCOL], BF16)
            nc.vector.tensor_mul(s_sb[:], psum1[:], mask_sb[:])

            pending.append((g, s_sb))
            if len(pending) > 3:
                emit_tail(*pending.pop(0))
        for item in pending:
            emit_tail(*item)

    nc.compile()
    return nc


def _prep_inputs(x, kernel, bias):
    """Pack full inputs into per-core tile-layout arrays (index shuffling only)."""
    x = np.ascontiguousarray(x, dtype=np.float32)
    kernel = np.ascontiguousarray(kernel, dtype=np.float32)
    bias = np.ascontiguousarray(bias, dtype=np.float32).reshape(L, F)

    # im2col: patches[b, l, p] with p=(kd,kh,kw,cin), matching the reference
    sw = np.lib.stride_tricks.sliding_window_view(x, (KD, KH, KW), axis=(1, 2, 3))
    # sw: (B, OD, OH, OW, Cin, KD, KH, KW) -> (B, L, KD,KH,KW,Cin) -> (B, L, P)
    patches = sw.transpose(0, 1, 2, 3, 5, 6, 7, 4).reshape(B, L, P)

    mask_np = np.zeros((MROW, NCOL), dtype=NP_BF16)
    for l in range(G):
        mask_np[B * l:B * l + B, F * l:F * l + F] = 1.0 / KSCALE
    # sel[:, q*EROW + (4q+b)] routes group q's b-rows into psum2 slot 4q+b
    sel_np = np.zeros((MROW, EP, EROW), dtype=NP_BF16)
    for q in range(EP):
        for b in range(B):
            sel_np[b::B, q, B * q + b] = 1.0
    sel_np = sel_np.reshape(MROW, EP * EROW)

    in_maps = []
    for c in range(NCORE):
        lo = c * LC
        kp = np.zeros((LP, P, F), dtype=np.float32)
        kp[:LC] = kernel[lo:lo + LC] * KSCALE
        bp = np.zeros((LP, F), dtype=np.float32)
        bp[:LC] = bias[lo:lo + LC] * KSCALE
        ap_ = np.zeros((B, LP, P), dtype=np.float32)
        ap_[:, :LC] = patches[:, lo:lo + LC]

        # rhs: R[s, p, j*NCOL + l*F + f] = kp[s*SG*G + j*G + l, p, f] (scaled)
        r = kp.reshape(NSUP, SG, G, P, F).transpose(0, 3, 1, 2, 4).reshape(
            NSUP, P, SG * NCOL)
        biasrow = bp.reshape(NSUP, 1, SG * NCOL)
        r1 = np.ascontiguousarray(r[:, :K1]).astype(NP_F8E3)
        r2 = np.zeros((NSUP, K2, SG * NCOL), dtype=NP_F8E3)
        r2[:, :P - K1] = r[:, K1:].astype(NP_F8E3)
        r2[:, P - K1] = biasrow[:, 0].astype(NP_F8E3)

        # lhsT: T[s, p, j*MROW + l*B + b] = patches[b, s*SG*G + j*G + l, p]
        t = ap_.reshape(B, NSUP, SG, G, P).transpose(1, 4, 2, 3, 0).reshape(
            NSUP, P, SG * MROW)
        t1 = np.ascontiguousarray(t[:, :K1]).astype(NP_BF16)
        t2 = np.zeros((NSUP, K2, SG * MROW), dtype=NP_F8E3)
        t2[:, :P - K1] = t[:, K1:]
        t2[:, P - K1] = 1.0

        in_maps.append(dict(r1=r1, r2=r2, t1=t1, t2=t2, mask=mask_np, sel=sel_np,
                            tok=np.zeros((1, 16), dtype=np.float32)))
    return in_maps


def _unpack_output(results):
    """results: list of per-core dicts with 'out' [NEP, SROW, NCOL]."""
    slabs = []
    for c in range(NCORE):
        o = np.asarray(results[c]["out"], dtype=np.float32)
        o = o[:, :EROW].reshape(NEP, EP, B, G, F)
        o = o.transpose(2, 0, 1, 3, 4).reshape(B, LP, F)[:, :LC]
        slabs.append(o)
    full = np.concatenate(slabs, axis=1)          # (B, L, F)
    return np.ascontiguousarray(full.reshape(B, OD, OH, OW, F))


def kernel(x, kernel, bias, _trace=False):
    if "nc" not in _CACHE:
        _CACHE["nc"] = _build()
    nc = _CACHE["nc"]
    in_maps = _prep_inputs(x, kernel, bias)
    res = bass_utils.run_bass_kernel_spmd(
        nc, in_maps, core_ids=list(range(NCORE)),
        trace=_trace, trace_cores=list(range(NCORE)) if _trace else None,
        stitch_traces=False,
    )
    out = _unpack_output(res.results)
    if _trace:
        return out, res
    return out


# revision 39
# speedup vs baseline: 1.0000x; 1.0000x over previous
"""LocallyConnected3D (valid, stride 1) as a Trainium2 Bass kernel on 8 NeuronCores.

Math: out[b,l,f] = sum_p patch[b,l,p] * K[l,p,f] + bias[l,f]
  with B=4, L=27000 output locations, P=216 receptive field, F=16 filters.

The (L,P,F) kernel tensor (373 MB fp32) dominates: each element is used
exactly B=4 times, so the problem is HBM-bandwidth-bound.  Strategy:

- Shard L across the 8 cores (3375 locations each, padded to 3456 = 108
  groups of 32 locations).
- Stream the kernel in fp8-e3m4 scaled by 64 (values land in e3m4's normal
  range; the 1/64 descale is folded into the block-diagonal mask constant)
  and the patches in bf16.  Measured end-to-end rel err ~1.5e-2 against the
  2e-2 gate (verified offline in numpy on the exact inputs).
- Per group g (32 locations), the kernel slab is the *moving* matmul operand:
  psum1[m=(l',b), n=(l,f)] = sum_p patch[b,l',p]*K[l,p,f] via two accumulating
  matmuls (contraction split p=[0,128) and p=[128,216) + one all-ones lhsT row
  carrying 64*bias[l,f]).  Only the block-diagonal l'==l entries are wanted;
  the PE redundancy is free because the kernel byte stream is the bottleneck.
- DVE multiplies psum1 by the bf16 mask (0 or 1/64) into bf16,
- a second matmul (MM2) with a rotating [128,108] selection matrix sums over
  l' AND routes group g's dense [4,512] result into partition slot
  4*(g%27)..+4 of a shared [108,512] PSUM tile, accumulating over a 27-group
  epoch.  One 128-lane scalar-engine copy and one 112-partition output DMA
  per epoch replace 27 lane-starved 4-partition copies/DMAs (4-partition DMAs
  serialize on a single SDMA engine and were the measured critical path).

DMA details (all HW-measured on this part):
- The HW descriptor-generation engine spreads a transfer across all 16 SDMA
  engines ONLY when the SBUF partition count is a multiple of 16; an
  89-partition transfer lands on a single engine and serializes everything.
  Hence chunk 2 is padded to 96 partitions and the staging tile to 48.
- Per-packet (per partition-row) overhead dominates below ~4 KB rows, so
  super-tiles cover SG=12 groups: kernel rows are 6 KB (fp8) and patch rows
  3 KB (bf16), with only 4 streaming transfers per super-tile.

Host-side numpy does the im2col patch extraction and packs kernel/patches
into the exact SBUF tile layouts (index shuffling + dtype cast only - all
FLOPs happen on device).
"""

from contextlib import ExitStack

import ml_dtypes
import numpy as np

import concourse.bacc as bacc
import concourse.mybir as mybir
import concourse.tile as tile
from concourse import bass_utils

F32 = mybir.dt.float32
BF16 = mybir.dt.bfloat16
F8E3 = mybir.dt.float8e3
NP_BF16 = ml_dtypes.bfloat16
NP_F8E3 = ml_dtypes.float8_e3m4

# Geometry (hardcoded per the problem spec)
B, D, H, W, Cin = 4, 32, 32, 32, 8
KD = KH = KW = 3
F = 16
OD = OH = OW = 30
L = OD * OH * OW           # 27000
P = KD * KH * KW * Cin     # 216
NCORE = 8
LC = L // NCORE            # 3375 locations per core
G = 32                     # locations per group
NG = 108                   # groups per core (LC padded to 3456)
LP = NG * G                # 3456
SG = 12                    # groups per DMA super-tile
NSUP = NG // SG            # 9
EP = 27                    # groups per MM2 accumulation epoch
NEP = NG // EP             # 4 output stages
EROW = EP * B              # 108 psum2 partitions per epoch
SROW = 112                 # staged rows (EROW padded to a multiple of 16)
K1 = 128                   # contraction chunk 1 (p in [0,128))
K2 = 96                    # chunk 2: 88 rows p in [128,216) + bias row + 7 pad
KSCALE = 64.0              # fp8 range scale for the kernel stream
NCOL = G * F               # 512 rhs columns per group
MROW = G * B               # 128 psum rows per group

_CACHE = {}


def _build(reps=1):
    nc = bacc.Bacc("TRN2", target_bir_lowering=False, debug=False)

    r1 = nc.dram_tensor("r1", [NSUP, K1, SG * NCOL], F8E3, kind="ExternalInput")
    r2 = nc.dram_tensor("r2", [NSUP, K2, SG * NCOL], F8E3, kind="ExternalInput")
    t1 = nc.dram_tensor("t1", [NSUP, K1, SG * MROW], BF16, kind="ExternalInput")
    t2 = nc.dram_tensor("t2", [NSUP, K2, SG * MROW], BF16, kind="ExternalInput")
    mask = nc.dram_tensor("mask", [MROW, NCOL], BF16, kind="ExternalInput")
    sel = nc.dram_tensor("sel", [MROW, EP * EROW], BF16, kind="ExternalInput")
    out = nc.dram_tensor("out", [NEP, SROW, NCOL], BF16, kind="ExternalOutput")
    # tiny passthrough token so a timing harness can chain executions
    tok = nc.dram_tensor("tok", [1, 16], F32, kind="ExternalInput")
    tok_out = nc.dram_tensor("tok_out", [1, 16], F32, kind="ExternalOutput")

    with tile.TileContext(nc) as tc, ExitStack() as ctx:
        const_pool = ctx.enter_context(tc.tile_pool(name="const", bufs=1))
        sup_pool = ctx.enter_context(tc.tile_pool(name="sup", bufs=4))
        s_pool = ctx.enter_context(tc.tile_pool(name="s", bufs=5))
        stage_pool = ctx.enter_context(tc.tile_pool(name="stage", bufs=2))
        ps1_pool = ctx.enter_context(tc.tile_pool(name="ps1", bufs=5, space="PSUM"))
        ps2_pool = ctx.enter_context(tc.tile_pool(name="ps2", bufs=2, space="PSUM"))

        # Constants ride the otherwise-idle GpSimd (SWDGE) ring so super 0's
        # kernel/patch streams lead the sync and scalar rings from t=0.
        mask_sb = const_pool.tile([MROW, NCOL], BF16)
        nc.gpsimd.dma_start(mask_sb[:], mask.ap())
        sel_sb = const_pool.tile([MROW, EP * EROW], BF16)
        nc.gpsimd.dma_start(sel_sb[:], sel.ap())
        tok_sb = const_pool.tile([1, 16], F32)
        nc.gpsimd.dma_start(tok_sb[:], tok.ap())
        nc.gpsimd.dma_start(tok_out.ap(), tok_sb[:])

        # Pre-warm the tensor engine while the first super-tile streams in:
        # ~20 dependency-free matmuls on a zeroed scratch tile take the PE
        # through its cold->warm ramp (HAM throttle) so real work starts at
        # full speed.  Results land in a scratch PSUM bank, never read.
        warm_pool = ctx.enter_context(tc.tile_pool(name="warm", bufs=1, space="PSUM"))
        warm_sb = const_pool.tile([MROW, NCOL], BF16)
        nc.vector.memset(warm_sb[:], 0.0)
        psum_w = warm_pool.tile([MROW, NCOL], F32)
        for _ in range(20):
            nc.tensor.matmul(psum_w[:], warm_sb[:, :MROW], warm_sb[:],
                             start=True, stop=True)

        sup = {}
        psum2 = None

        def emit_tail(g, s_sb):
            # MM2: psum2[4q+b, (l,f)] += sum_{l'} S[(l',b), (l,f)] for q=g%EP,
            # accumulating an epoch of EP groups into one [EROW, NCOL] tile.
            nonlocal psum2
            e, q = g // EP, g % EP
            if q == 0:
                psum2 = ps2_pool.tile([EROW, NCOL], F32)
            nc.tensor.matmul(psum2[:], sel_sb[:, q * EROW:(q + 1) * EROW], s_sb[:],
                             start=(q == 0), stop=(q == EP - 1))
            if q == EP - 1:
                stage_sb = stage_pool.tile([SROW, NCOL], BF16)
                nc.scalar.copy(stage_sb[:EROW, :], psum2[:])
                nc.sync.dma_start(out.ap()[e], stage_sb[:])

        pending = []
        for g_rep in range(reps * NG):
            g = g_rep % NG
            s, j = g // SG, g % SG
            if j == 0:
                sup["r1"] = sup_pool.tile([K1, SG * NCOL], F8E3, tag="r1", name="r1sb")
                nc.sync.dma_start(sup["r1"][:], r1.ap()[s])
                sup["r2"] = sup_pool.tile([K2, SG * NCOL], F8E3, tag="r2", name="r2sb")
                nc.sync.dma_start(sup["r2"][:], r2.ap()[s])
                sup["t1"] = sup_pool.tile([K1, SG * MROW], BF16, tag="t1", name="t1sb")
                nc.scalar.dma_start(sup["t1"][:], t1.ap()[s])
                sup["t2"] = sup_pool.tile([K2, SG * MROW], BF16, tag="t2", name="t2sb")
                nc.scalar.dma_start(sup["t2"][:], t2.ap()[s])

            psum1 = ps1_pool.tile([MROW, NCOL], F32)
            nc.tensor.matmul(
                psum1[:],
                sup["t1"][:, j * MROW:(j + 1) * MROW],
                sup["r1"][:, j * NCOL:(j + 1) * NCOL],
                start=True, stop=False,
            )
            nc.tensor.matmul(
                psum1[:],
                sup["t2"][:, j * MROW:(j + 1) * MROW],
                sup["r2"][:, j * NCOL:(j + 1) * NCOL],
                start=False, stop=True,
            )
            s_sb = s_pool.tile([MROW, NCOL], BF16)
            nc.vector.tensor_mul(s_sb[:], psum1[:], mask_sb[:])

            pending.append((g, s_sb))
            if len(pending) > 3:
                emit_tail(*pending.pop(0))
        for item in pending:
            emit_tail(*item)

    nc.compile()
    return nc


def _prep_inputs(x, kernel, bias):
    """Pack full inputs into per-core tile-layout arrays (index shuffling only)."""
    x = np.ascontiguousarray(x, dtype=np.float32)
    kernel = np.ascontiguousarray(kernel, dtype=np.float32)
    bias = np.ascontiguousarray(bias, dtype=np.float32).reshape(L, F)

    # im2col: patches[b, l, p] with p=(kd,kh,kw,cin), matching the reference
    sw = np.lib.stride_tricks.sliding_window_view(x, (KD, KH, KW), axis=(1, 2, 3))
    # sw: (B, OD, OH, OW, Cin, KD, KH, KW) -> (B, L, KD,KH,KW,Cin) -> (B, L, P)
    patches = sw.transpose(0, 1, 2, 3, 5, 6, 7, 4).reshape(B, L, P)

    mask_np = np.zeros((MROW, NCOL), dtype=NP_BF16)
    for l in range(G):
        mask_np[B * l:B * l + B, F * l:F * l + F] = 1.0 / KSCALE
    # sel[:, q*EROW + (4q+b)] routes group q's b-rows into psum2 slot 4q+b
    sel_np = np.zeros((MROW, EP, EROW), dtype=NP_BF16)
    for q in range(EP):
        for b in range(B):
            sel_np[b::B, q, B * q + b] = 1.0
    sel_np = sel_np.reshape(MROW, EP * EROW)

    in_maps = []
    for c in range(NCORE):
        lo = c * LC
        kp = np.zeros((LP, P, F), dtype=np.float32)
        kp[:LC] = kernel[lo:lo + LC] * KSCALE
        bp = np.zeros((LP, F), dtype=np.float32)
        bp[:LC] = bias[lo:lo + LC] * KSCALE
        ap_ = np.zeros((B, LP, P), dtype=np.float32)
        ap_[:, :LC] = patches[:, lo:lo + LC]

        # rhs: R[s, p, j*NCOL + l*F + f] = kp[s*SG*G + j*G + l, p, f] (scaled)
        r = kp.reshape(NSUP, SG, G, P, F).transpose(0, 3, 1, 2, 4).reshape(
            NSUP, P, SG * NCOL)
        biasrow = bp.reshape(NSUP, 1, SG * NCOL)
        r1 = np.ascontiguousarray(r[:, :K1]).astype(NP_F8E3)
        r2 = np.zeros((NSUP, K2, SG * NCOL), dtype=NP_F8E3)
        r2[:, :P - K1] = r[:, K1:].astype(NP_F8E3)
        r2[:, P - K1] = biasrow[:, 0].astype(NP_F8E3)

        # lhsT: T[s, p, j*MROW + l*B + b] = patches[b, s*SG*G + j*G + l, p]
        t = ap_.reshape(B, NSUP, SG, G, P).transpose(1, 4, 2, 3, 0).reshape(
            NSUP, P, SG * MROW)
        t1 = np.ascontiguousarray(t[:, :K1]).astype(NP_BF16)
        t2 = np.zeros((NSUP, K2, SG * MROW), dtype=NP_BF16)
        t2[:, :P - K1] = t[:, K1:]
        t2[:, P - K1] = 1.0

        in_maps.append(dict(r1=r1, r2=r2, t1=t1, t2=t2, mask=mask_np, sel=sel_np,
                            tok=np.zeros((1, 16), dtype=np.float32)))
    return in_maps


def _unpack_output(results):
    """results: list of per-core dicts with 'out' [NEP, SROW, NCOL]."""
    slabs = []
    for c in range(NCORE):
        o = np.asarray(results[c]["out"], dtype=np.float32)
        o = o[:, :EROW].reshape(NEP, EP, B, G, F)
        o = o.transpose(2, 0, 1, 3, 4).reshape(B, LP, F)[:, :LC]
        slabs.append(o)
    full = np.concatenate(slabs, axis=1)          # (B, L, F)
    return np.ascontiguousarray(full.reshape(B, OD, OH, OW, F))


def kernel(x, kernel, bias, _trace=False):
    if "nc" not in _CACHE:
        _CACHE["nc"] = _build()
    nc = _CACHE["nc"]
    in_maps = _prep_inputs(x, kernel, bias)
    res = bass_utils.run_bass_kernel_spmd(
        nc, in_maps, core_ids=list(range(NCORE)),
        trace=_trace, trace_cores=list(range(NCORE)) if _trace else None,
        stitch_traces=False,
    )
    out = _unpack_output(res.results)
    if _trace:
        return out, res
    return out
